# revision 1
# baseline (speedup 1.0000x reference)
"""Trainium2 Bass kernel for nn_ButterflyLayer1D.

Data-parallel across 8 NeuronCores: each core processes 128 of the 1024
samples; the butterfly filter tree is replicated to every core.

Per-core layout convention: activations live in SBUF as
(channels=128 partitions, free = [branch..., position..., sample(128)])
with samples innermost, so every matmul is a K=128 x M=128 weight applied
to 512-column tiles of the 8192-column activation plane.  All nine stages
(input conv, 3 down levels, middle switch, 3 up levels, output conv)
output exactly 8192 columns x 128 channels per core.

Matmuls run in bf16 (weights and activations; full-rate 1 col/cycle on the
PE array) with fp32 PSUM accumulation.  Per-branch biases are applied by
the Scalar/Vector engine epilogues (relu + bias from PSUM, two 1024-col
ops per psum tile on opposite engines).  The middle switch has a distinct
bias per 128-col block, which is instead seeded into PSUM by a K=4
indicator matmul before the per-(itk,itx) c-x-c matmuls accumulate on top.
"""

import sys

for _p in ("/opt/trn_rl_repo",):
    if _p not in sys.path:
        sys.path.insert(0, _p)

import numpy as np
import ml_dtypes

import concourse.bass as bass
import concourse.bacc as bacc
import concourse.mybir as mybir
from concourse.tile import TileContext
from concourse.bass_utils import run_bass_kernel_spmd

C = 128            # channels == partitions == contraction size
N_CORES = 8
NPC = 128          # samples per core
NCOL = 64 * NPC    # 8192 free columns per stage
F32 = mybir.dt.float32
BF16 = mybir.dt.bfloat16
AF = mybir.ActivationFunctionType
ALU = mybir.AluOpType

PT = 1024          # psum tile columns (2 banks); 4 tiles fill PSUM
SUB = 512          # matmul moving-operand columns


def build_nc():
    nc = bacc.Bacc(enable_partition_id=False)

    dp = lambda name, shape, dt=BF16: nc.declare_dram_parameter(name, list(shape), dt, False)
    xt_d = dp("xt", (C, NCOL))
    wxf_d = dp("wxf", (C, C))
    w123_d = dp("w123", (C, 28 * C))      # [w1 | w2 | w3]
    wm_d = dp("wm", (C, 64 * C))
    w456k_d = dp("w456k", (C, 29 * C))    # [w4 | w5 | w6 | wkf]
    bia_d = dp("bia", (C, 93), F32)       # [xb|b1|b2|b3|b4|b5|b6|mb]
    mb2_d = dp("mb2", (4, 16 * C))        # mid biases as K=4 lhsT slices
    ind_d = dp("ind", (4, 512))           # 4x512 block indicator
    out_d = nc.declare_dram_parameter("out", [C, NCOL], BF16, True)

    from contextlib import ExitStack

    with TileContext(nc) as tc, ExitStack() as ctx:
        singles = ctx.enter_context(tc.tile_pool(name="weights", bufs=1))
        act_pool = ctx.enter_context(tc.tile_pool(name="act", bufs=2))
        psum_pool = ctx.enter_context(tc.tile_pool(name="psum", bufs=4, space="PSUM"))

        def load(dram, shape, dt=BF16, split=1, name=None):
            t = singles.tile(list(shape), dt, tag=name, name=name)
            step = shape[1] // split
            for i in range(split):
                nc.sync.dma_start(
                    out=t[:, i * step : (i + 1) * step],
                    in_=dram[:, i * step : (i + 1) * step],
                )
            return t

        # Critical-path loads first: stage-0 weights + biases + the first x
        # columns, then the rest of x and the deeper weights.
        xt = singles.tile([C, NCOL], BF16, tag="xt_sb", name="xt_sb")
        nc.sync.dma_start(out=xt[:, 0:512], in_=xt_d[:, 0:512])
        wxf = load(wxf_d, (C, C), name="wxf_sb")
        bia = load(bia_d, (C, 93), dt=F32, name="bia_sb")
        nc.scalar.dma_start(out=xt[:, 512:1536], in_=xt_d[:, 512:1536])
        nc.sync.dma_start(out=xt[:, 1536:4096], in_=xt_d[:, 1536:4096])
        nc.scalar.dma_start(out=xt[:, 4096:8192], in_=xt_d[:, 4096:8192])
        w123 = load(w123_d, (C, 28 * C), name="w123_sb")
        wm = load(wm_d, (C, 64 * C), split=2, name="wm_sb")
        w456k = load(w456k_d, (C, 29 * C), name="w456k_sb")
        mb2 = load(mb2_d, (4, 16 * C), name="mb2_sb")
        ind = load(ind_d, (4, 512), name="ind_sb")
        w1, w2, w3 = w123[:, : 4 * C], w123[:, 4 * C : 12 * C], w123[:, 12 * C : 28 * C]
        w4, w5 = w456k[:, : 16 * C], w456k[:, 16 * C : 24 * C]
        w6, wkf = w456k[:, 24 * C : 28 * C], w456k[:, 28 * C : 29 * C]
        xb, b1, b2 = bia[:, 0:1], bia[:, 1:3], bia[:, 3:7]
        b3, b4, b5 = bia[:, 7:15], bia[:, 15:23], bia[:, 23:27]
        b6, mb = bia[:, 27:29], bia[:, 29:93]

        load_ns = {"s": 0.0, "v": 0.0}

        def epi(out_ap, in_ap, bias_ap, relu=True, cols=PT):
            """One epilogue op: out = relu(in + bias) (or copy); greedy engine balance."""
            cost = {"s": (352 + cols) / 1.2, "v": (120 + cols) / 0.96}
            eng = "s" if load_ns["s"] + cost["s"] <= load_ns["v"] + cost["v"] else "v"
            load_ns[eng] += cost[eng]
            if bias_ap is None and not relu:
                if eng == "s":
                    nc.scalar.activation(out_ap, in_ap, AF.Copy)
                else:
                    nc.vector.tensor_copy(out_ap, in_ap)
            elif bias_ap is None:
                if eng == "s":
                    nc.scalar.activation(out_ap, in_ap, AF.Relu)
                else:
                    nc.vector.tensor_scalar_max(out_ap, in_ap, 0.0)
            else:
                if eng == "s":
                    nc.scalar.activation(out_ap, in_ap, AF.Relu, bias=bias_ap)
                else:
                    nc.vector.tensor_scalar(out_ap, in_ap, bias_ap, 0.0, ALU.add, ALU.max)

        # ---------------- stage 0: input conv ----------------
        v0 = act_pool.tile([C, NCOL], BF16, tag="act", name="v0")

        def s0_tiles(ts):
            for t in ts:
                pt = psum_pool.tile([C, PT], F32, tag="pt", name="p0")
                for s in range(2):
                    col = t * PT + s * SUB
                    nc.tensor.matmul(
                        pt[:, s * SUB : (s + 1) * SUB],
                        wxf[:, :],
                        xt[:, col : col + SUB],
                        start=True,
                        stop=True,
                    )
                epi(v0[:, t * PT : (t + 1) * PT], pt[:, :], xb[:, 0:1], cols=PT)

        # ---------------- down levels 1..3 ----------------
        def down_level(vin, vout, w_sb, b_sb, nb_out, l_out, tiles=None):
            """vin: (c, [nb_in, 2*l_out, n]); vout: (c, [nb_out, l_out, n])."""
            wv = w_sb.rearrange("p (b k d) -> p b k d", b=nb_out, k=2, d=C)
            vi = vin.rearrange("p (b l k n) -> p b l k n", b=nb_out // 2, l=l_out, k=2, n=NPC)
            vo = vout.rearrange("p (b l n) -> p b l n", b=nb_out, l=l_out, n=NPC)
            cpb = l_out * NPC  # columns per output branch (>= 1024 for levels 1..3)
            for t in tiles if tiles is not None else range(NCOL // PT):
                pt = psum_pool.tile([C, PT], F32, tag="pt", name="pd")
                for k in range(2):
                    for s in range(2):
                        col = t * PT + s * SUB
                        b = col // cpb
                        l0 = (col % cpb) // NPC
                        nc.tensor.matmul(
                            pt[:, s * SUB : (s + 1) * SUB],
                            wv[:, b, k, :],
                            vi[:, b // 2, l0 : l0 + SUB // NPC, k, :],
                            start=(k == 0),
                            stop=(k == 1),
                        )
                b = (t * PT) // cpb
                l0 = ((t * PT) % cpb) // NPC
                epi(
                    vo[:, b, l0 : l0 + PT // NPC, :],
                    pt[:, :],
                    b_sb[:, b : b + 1],
                    cols=PT,
                )

        v1 = act_pool.tile([C, NCOL], BF16, tag="act", name="v1")
        s0_tiles((0, 1))
        for g in range(4):
            if g < 3:
                s0_tiles((2 * g + 2, 2 * g + 3))
            down_level(v0, v1, w1, b1, 2, 32, tiles=(g, g + 4))
        v2 = act_pool.tile([C, NCOL], BF16, tag="act", name="v2")
        down_level(v1, v2, w2, b2, 4, 16)
        v3 = act_pool.tile([C, NCOL], BF16, tag="act", name="v3")
        down_level(v2, v3, w3, b3, 8, 8)

        # ---------------- middle switch ----------------
        # v3: (c, [itk=8, itx=8, n]); vm: (c, [itx=8, itk=8, n])
        # Per-(itx,itk) bias seeded into PSUM by a K=4 indicator matmul,
        # then the per-block c x c matmuls accumulate on top.
        vm = act_pool.tile([C, NCOL], BF16, tag="act", name="vm")
        v3v = v3.rearrange("p (k x n) -> p k x n", k=8, x=8, n=NPC)
        wmv = wm.rearrange("p (k x d) -> p k x d", k=8, x=8, d=C)
        for t in range(NCOL // PT):  # tile t covers itx = t
            pt = psum_pool.tile([C, PT], F32, tag="pt", name="pm")
            if t % 2 == 0:
                # seed per-block biases into PSUM with K=4 indicator matmuls,
                # then accumulate the per-(itk,itx) c x c matmuls on top; the
                # epilogue is then a plain relu (greedy engine choice).
                for sgrp in range(2):
                    nc.tensor.matmul(
                        pt[:, sgrp * SUB : (sgrp + 1) * SUB],
                        mb2[:, (2 * t + sgrp) * C : (2 * t + sgrp + 1) * C],
                        ind[:, :],
                        start=True,
                        stop=False,
                        skip_group_check=True,
                    )
                    for bi in range(4):
                        blk = 4 * sgrp + bi
                        nc.tensor.matmul(
                            pt[:, blk * NPC : (blk + 1) * NPC],
                            wmv[:, blk, t, :],
                            v3v[:, blk, t, :],
                            start=False,
                            stop=(bi == 3),
                            skip_group_check=True,
                        )
                epi(vm[:, t * PT : (t + 1) * PT], pt[:, :], None, cols=PT)
            else:
                for blk in range(8):  # block within tile (= itk); global = 8t + blk
                    nc.tensor.matmul(
                        pt[:, blk * NPC : (blk + 1) * NPC],
                        wmv[:, blk, t, :],
                        v3v[:, blk, t, :],
                        start=True,
                        stop=True,
                    )
                # TT-add (V) with a broadcast bias view + in-place Scalar relu
                ptv = pt.rearrange("p (b n) -> p b n", b=8, n=NPC)
                bias_v = mb[:, 8 * t : 8 * (t + 1)].unsqueeze(2).broadcast_to((C, 8, NPC))
                dst = vm[:, t * PT : (t + 1) * PT]
                dstv = dst.rearrange("p (b n) -> p b n", b=8, n=NPC)
                nc.vector.tensor_tensor(dstv, ptv, bias_v, ALU.add)
                load_ns["v"] += (120 + PT) / 0.96
                nc.scalar.activation(dst, dst, AF.Relu)
                load_ns["s"] += (352 + PT) / 1.2

        # ---------------- up levels 4..6 ----------------
        def up_level(vin, vout, w_sb, b_sb, nb_in, l_in, tiles=None):
            """vin: (c, [x=nb_in, l_in, n]); vout: (c, [xo=nb_in/2, 2*l_in, n]);
            vout[:, xo, 2*l+j, :] = relu(sum_k vin[:, 2xo+k, l, :] @ W[xo,j,k] + B[xo,j])."""
            nbo = nb_in // 2
            wv = w_sb.rearrange("p (x j k d) -> p x j k d", x=nbo, j=2, k=2, d=C)
            vi = vin.rearrange("p (x l n) -> p x l n", x=nb_in, l=l_in, n=NPC)
            vo = vout.rearrange("p (x l j n) -> p x l j n", x=nbo, l=l_in, j=2, n=NPC)
            cpb = l_in * NPC  # columns per (xo, j) output block
            for t in tiles if tiles is not None else range(NCOL // PT):
                pt = psum_pool.tile([C, PT], F32, tag="pt", name="pu")
                for k in range(2):
                    for s in range(2):
                        col = t * PT + s * SUB
                        g = col // cpb  # global (xo, j) block index, j-minor
                        xo, j = g // 2, g % 2
                        lt0 = (col % cpb) // NPC
                        nc.tensor.matmul(
                            pt[:, s * SUB : (s + 1) * SUB],
                            wv[:, xo, j, k, :],
                            vi[:, 2 * xo + k, lt0 : lt0 + SUB // NPC, :],
                            start=(k == 0),
                            stop=(k == 1),
                        )
                g = (t * PT) // cpb
                xo, j = g // 2, g % 2
                lt0 = ((t * PT) % cpb) // NPC
                epi(
                    vo[:, xo, lt0 : lt0 + PT // NPC, j, :],
                    pt[:, :],
                    b_sb[:, 2 * xo + j : 2 * xo + j + 1],
                    cols=PT,
                )

        v4 = act_pool.tile([C, NCOL], BF16, tag="act", name="v4")
        up_level(vm, v4, w4, b4, 8, 8)
        v5 = act_pool.tile([C, NCOL], BF16, tag="act", name="v5")
        up_level(v4, v5, w5, b5, 4, 16)
        v6 = act_pool.tile([C, NCOL], BF16, tag="act", name="v6")
        yo = singles.tile([C, NCOL], BF16, tag="yo_sb", name="yo")

        # ---------------- output conv (no bias / relu), interleaved with L6 --
        def out_tiles(ts):
            for t in ts:
                pt = psum_pool.tile([C, PT], F32, tag="pt", name="po")
                for s in range(2):
                    col = t * PT + s * SUB
                    nc.tensor.matmul(
                        pt[:, s * SUB : (s + 1) * SUB],
                        wkf[:, :],
                        v6[:, col : col + SUB],
                        start=True,
                        stop=True,
                    )
                epi(yo[:, t * PT : (t + 1) * PT], pt[:, :], None, relu=False, cols=PT)
                deng = nc.sync if t % 2 == 0 else nc.scalar
                deng.dma_start(
                    out=out_d[:, t * PT : (t + 1) * PT],
                    in_=yo[:, t * PT : (t + 1) * PT],
                )

        # L6 j=0 tiles are 0..3, j=1 tiles are 4..7 (cpb=4096); out tile pair
        # (2q, 2q+1) needs quarter q of both j streams.  Run one quarter ahead
        # so out-tile matmuls never wait on a just-finished L6 epilogue.
        up_level(v5, v6, w6, b6, 2, 32, tiles=(0, 4))
        for q in range(3):
            up_level(v5, v6, w6, b6, 2, 32, tiles=(q + 1, 5 + q))
            out_tiles((2 * q, 2 * q + 1))
        out_tiles((6, 7))

    nc.finalize()
    return nc


_NC_CACHE = {}


def _get_nc():
    if "nc" not in _NC_CACHE:
        _NC_CACHE["nc"] = build_nc()
    return _NC_CACHE["nc"]


def _prep_in_maps(inputs):
    x = np.asarray(inputs["x"], np.float32)
    bf = lambda a: np.ascontiguousarray(np.asarray(a, np.float32)).astype(ml_dtypes.bfloat16)
    f32 = lambda a: np.ascontiguousarray(np.asarray(a, np.float32))
    mbv = np.asarray(inputs["mb"], np.float32)  # (k=8, x=8, c)
    mbT = mbv.transpose(1, 0, 2).reshape(64, C).T  # (c, 64), col = x*8 + k
    wmat = lambda key, nb: np.asarray(inputs[key], np.float32).reshape(nb, C, C).transpose(1, 0, 2).reshape(C, nb * C)
    w123 = np.concatenate([wmat("f1", 4), wmat("f2", 8), wmat("f3", 16)], axis=1)
    w456k = np.concatenate(
        [wmat("f4", 16), wmat("f5", 8), wmat("f6", 4), np.asarray(inputs["kf"], np.float32)], axis=1
    )
    bia = np.concatenate(
        [
            np.asarray(inputs["xb"], np.float32).reshape(C, 1),
            np.asarray(inputs["b1"], np.float32).T,
            np.asarray(inputs["b2"], np.float32).T,
            np.asarray(inputs["b3"], np.float32).T,
            np.asarray(inputs["b4"], np.float32).T,
            np.asarray(inputs["b5"], np.float32).T,
            np.asarray(inputs["b6"], np.float32).T,
            mbT,
        ],
        axis=1,
    )
    # mid-bias lhsT slices: u = 2*t + sgrp (t = itx tile, sgrp = 512-col half);
    # row ki covers block k = 4*sgrp + ki at x = t: mb2[ki, u*C+d] = mb[4*(u%2)+ki, u//2, d]
    mb2 = np.zeros((4, 16 * C), np.float32)
    for u in range(16):
        t_, sgrp = u // 2, u % 2
        for ki in range(4):
            mb2[ki, u * C : (u + 1) * C] = mbv[4 * sgrp + ki, t_, :]
    ind = np.zeros((4, 512), np.float32)
    for ki in range(4):
        ind[ki, ki * NPC : (ki + 1) * NPC] = 1.0
    shared = {
        "mb2": bf(mb2),
        "ind": bf(ind),
        "wxf": bf(inputs["xf"]),  # (f=128, c) as lhsT directly
        "w123": bf(w123),
        "wm": bf(np.asarray(inputs["md"], np.float32).reshape(64, C, C).transpose(1, 0, 2).reshape(C, 64 * C)),
        "w456k": bf(w456k),
        "bia": f32(bia),
    }
    in_maps = []
    for i in range(N_CORES):
        xs = x[i * NPC : (i + 1) * NPC]  # (128, 8192)
        xt = (
            np.ascontiguousarray(xs.reshape(NPC, 64, C).transpose(2, 1, 0))
            .reshape(C, NCOL)
            .astype(ml_dtypes.bfloat16)
        )
        in_maps.append({"xt": xt, **shared})
    return in_maps


def _gather(results):
    outs = []
    for i in range(N_CORES):
        r = np.asarray(results[i]["out"]).astype(np.float32)  # (C=k_out, [l=64, n=128])
        outs.append(r.reshape(C, 64, NPC).transpose(2, 1, 0).reshape(NPC, 64 * C))
    return np.concatenate(outs, axis=0).astype(np.float32)


def _enable_ntff_hook():
    """Register the axon NTFF profiling hook (missing from this image's
    antenv) so run_bass_kernel_spmd(trace=True) can measure HW exec time."""
    import types

    if "antenv.axon_hooks" in sys.modules:
        return
    import antenv
    from trn_agent_boot.trn_boot import _ntff_profile_via_ctypes

    hook = _ntff_profile_via_ctypes("/opt/axon/libaxon_pjrt.so")
    mod = types.ModuleType("antenv.axon_hooks")
    mod.get_axon_ntff_profile_hook = lambda: hook
    mod.set_axon_ntff_profile_hook = lambda h: None
    sys.modules["antenv.axon_hooks"] = mod
    antenv.axon_hooks = mod
    import concourse.bass_utils as bu

    bu.upload_artifacts = lambda tmpdir: tmpdir  # keep artifacts local


def run(inputs, trace=False, **kw):
    nc = _get_nc()
    in_maps = _prep_in_maps(inputs)
    if trace:
        _enable_ntff_hook()
    res = run_bass_kernel_spmd(nc, in_maps, core_ids=list(range(N_CORES)), trace=trace, **kw)
    return _gather(res.results), res


def kernel(**inputs) -> np.ndarray:
    out, _ = run(inputs, trace=False)
    return out



# revision 2
# speedup vs baseline: 1.0004x; 1.0004x over previous
"""Trainium2 Bass kernel for nn_ButterflyLayer1D.

Data-parallel across 8 NeuronCores: each core processes 128 of the 1024
samples; the butterfly filter tree is replicated to every core.

Per-core layout convention: activations live in SBUF as
(channels=128 partitions, free = [branch..., position..., sample(128)])
with samples innermost, so every matmul is a K=128 x M=128 weight applied
to 512-column tiles of the 8192-column activation plane.  All nine stages
(input conv, 3 down levels, middle switch, 3 up levels, output conv)
output exactly 8192 columns x 128 channels per core.

Matmuls run in bf16 (weights and activations; full-rate 1 col/cycle on the
PE array) with fp32 PSUM accumulation.  Per-branch biases are applied by
the Scalar/Vector engine epilogues (relu + bias from PSUM, two 1024-col
ops per psum tile on opposite engines).  The middle switch has a distinct
bias per 128-col block, which is instead seeded into PSUM by a K=4
indicator matmul before the per-(itk,itx) c-x-c matmuls accumulate on top.

I/O plan: all weights live in one DRAM blob loaded by four dma_starts in
consumption order (wxf | w123 | wm | w456k+wkf); x is loaded in five
consumption-ordered chunks (small first so stage 0 starts ~2us in).
Critical loads are split between the Sync and Scalar issue queues.
Output is stored by five chunked dma_starts issued as soon as each
block's epilogue lands.
"""

import sys

for _p in ("/opt/trn_rl_repo",):
    if _p not in sys.path:
        sys.path.insert(0, _p)

import numpy as np
import ml_dtypes

import concourse.bass as bass
import concourse.bacc as bacc
import concourse.mybir as mybir
from concourse.tile import TileContext
from concourse.bass_utils import run_bass_kernel_spmd

C = 128            # channels == partitions == contraction size
N_CORES = 8
NPC = 128          # samples per core
NCOL = 64 * NPC    # 8192 free columns per stage
F32 = mybir.dt.float32
BF16 = mybir.dt.bfloat16
AF = mybir.ActivationFunctionType
ALU = mybir.AluOpType

PT = 1024          # psum tile columns (2 banks); 4 tiles fill PSUM
SUB = 512          # matmul moving-operand columns

# column offsets inside the weight blob (C, 15616)
WB_COLS = 15616
OFF_XF = 0
OFF_W1 = 128
OFF_W2 = OFF_W1 + 4 * C
OFF_W3 = OFF_W2 + 8 * C
OFF_WM = OFF_W3 + 16 * C        # 3712
OFF_W4 = OFF_WM + 64 * C        # 11904
OFF_W5 = OFF_W4 + 16 * C
OFF_W6 = OFF_W5 + 8 * C
OFF_KF = OFF_W6 + 4 * C         # 15488

# xt load chunks (consumption order, small first for a fast start)
XT_CHUNKS_SYNC = ((0, 512), (1536, 3072), (3072, 5120), (5120, 8192))
XT_CHUNKS_SCAL = ((512, 1536),)
# output store chunks
OUT_CHUNKS = ((0, 2048), (2048, 4096), (4096, 6144), (6144, 7168), (7168, 8192))


def build_nc():
    nc = bacc.Bacc(enable_partition_id=False)

    dp = lambda name, shape, dt=BF16: nc.declare_dram_parameter(name, list(shape), dt, False)
    xt_d = dp("xt", (C, NCOL))
    wb_d = dp("wb", (C, WB_COLS))
    bia_d = dp("bia", (C, 93), F32)       # [xb|b1|b2|b3|b4|b5|b6|mb]
    mbind_d = dp("mbind", (4, 20 * C))    # [mb2 (16C) | ind (4C)]
    out_d = nc.declare_dram_parameter("out", [C, NCOL], BF16, True)

    from contextlib import ExitStack

    with TileContext(nc) as tc, ExitStack() as ctx:
        singles = ctx.enter_context(tc.tile_pool(name="weights", bufs=1))
        act_pool = ctx.enter_context(tc.tile_pool(name="act", bufs=3))
        psum_pool = ctx.enter_context(tc.tile_pool(name="psum", bufs=4, space="PSUM"))

        wb = singles.tile([C, WB_COLS], BF16, tag="wb_sb", name="wb_sb")
        xt = singles.tile([C, NCOL], BF16, tag="xt_sb", name="xt_sb")
        bia = singles.tile([C, 93], F32, tag="bia_sb", name="bia_sb")
        mbind = singles.tile([4, 20 * C], BF16, tag="mbind_sb", name="mbind_sb")
        yo = singles.tile([C, NCOL], BF16, tag="yo_sb", name="yo")

        # Critical-path loads first: wxf + bias + the first x columns, then
        # the rest of x and the deeper weights, all in consumption order.
        nc.sync.dma_start(out=wb[:, OFF_XF:OFF_W1], in_=wb_d[:, OFF_XF:OFF_W1])
        nc.scalar.dma_start(out=bia[:, :], in_=bia_d[:, :])
        nc.sync.dma_start(out=xt[:, 0:512], in_=xt_d[:, 0:512])
        nc.scalar.dma_start(out=xt[:, 512:1536], in_=xt_d[:, 512:1536])
        nc.sync.dma_start(out=wb[:, OFF_W1:OFF_WM], in_=wb_d[:, OFF_W1:OFF_WM])
        nc.scalar.dma_start(out=mbind[:, :], in_=mbind_d[:, :])
        for a, b in XT_CHUNKS_SYNC[1:]:
            nc.sync.dma_start(out=xt[:, a:b], in_=xt_d[:, a:b])
        nc.sync.dma_start(out=wb[:, OFF_WM:OFF_W4], in_=wb_d[:, OFF_WM:OFF_W4])
        nc.sync.dma_start(out=wb[:, OFF_W4:], in_=wb_d[:, OFF_W4:])

        wxf = wb[:, OFF_XF:OFF_W1]
        w1, w2 = wb[:, OFF_W1:OFF_W2], wb[:, OFF_W2:OFF_W3]
        w3, wm = wb[:, OFF_W3:OFF_WM], wb[:, OFF_WM:OFF_W4]
        w4, w5 = wb[:, OFF_W4:OFF_W5], wb[:, OFF_W5:OFF_W6]
        w6, wkf = wb[:, OFF_W6:OFF_KF], wb[:, OFF_KF:]
        mb2, ind = mbind[:, : 16 * C], mbind[:, 16 * C : 16 * C + 512]
        xb, b1, b2 = bia[:, 0:1], bia[:, 1:3], bia[:, 3:7]
        b3, b4, b5 = bia[:, 7:15], bia[:, 15:23], bia[:, 23:27]
        b6, mb = bia[:, 27:29], bia[:, 29:93]

        load_ns = {"s": 2500.0, "v": 0.0}  # scalar starts busy with DMA issue

        def epi(out_ap, in_ap, bias_ap, relu=True, cols=PT):
            """One epilogue op: out = relu(in + bias) (or copy); greedy engine balance."""
            cost = {"s": (352 + cols) / 1.2, "v": (120 + cols) / 0.96}
            eng = "s" if load_ns["s"] + cost["s"] <= load_ns["v"] + cost["v"] else "v"
            load_ns[eng] += cost[eng]
            if bias_ap is None and not relu:
                if eng == "s":
                    nc.scalar.activation(out_ap, in_ap, AF.Copy)
                else:
                    nc.vector.tensor_copy(out_ap, in_ap)
            elif bias_ap is None:
                if eng == "s":
                    nc.scalar.activation(out_ap, in_ap, AF.Relu)
                else:
                    nc.vector.tensor_scalar_max(out_ap, in_ap, 0.0)
            else:
                if eng == "s":
                    nc.scalar.activation(out_ap, in_ap, AF.Relu, bias=bias_ap)
                else:
                    nc.vector.tensor_scalar(out_ap, in_ap, bias_ap, 0.0, ALU.add, ALU.max)

        # ---------------- stage 0: input conv ----------------
        v0 = act_pool.tile([C, NCOL], BF16, tag="act", name="v0")

        def s0_tiles(ts):
            for t in ts:
                pt = psum_pool.tile([C, PT], F32, tag="pt", name="p0")
                for s in range(2):
                    col = t * PT + s * SUB
                    nc.tensor.matmul(
                        pt[:, s * SUB : (s + 1) * SUB],
                        wxf[:, :],
                        xt[:, col : col + SUB],
                        start=True,
                        stop=True,
                    )
                epi(v0[:, t * PT : (t + 1) * PT], pt[:, :], xb[:, 0:1], cols=PT)

        # ---------------- down levels 1..3 ----------------
        def down_level(vin, vout, w_sb, b_sb, nb_out, l_out, tiles=None):
            """vin: (c, [nb_in, 2*l_out, c]); vout: (c, [nb_out, l_out, c])."""
            wv = w_sb.rearrange("p (b k d) -> p b k d", b=nb_out, k=2, d=C)
            vi = vin.rearrange("p (b l k n) -> p b l k n", b=nb_out // 2, l=l_out, k=2, n=NPC)
            vo = vout.rearrange("p (b l n) -> p b l n", b=nb_out, l=l_out, n=NPC)
            cpb = l_out * NPC  # columns per output branch (>= 1024 for levels 1..3)
            for t in tiles if tiles is not None else range(NCOL // PT):
                pt = psum_pool.tile([C, PT], F32, tag="pt", name="pd")
                for k in range(2):
                    for s in range(2):
                        col = t * PT + s * SUB
                        b = col // cpb
                        l0 = (col % cpb) // NPC
                        nc.tensor.matmul(
                            pt[:, s * SUB : (s + 1) * SUB],
                            wv[:, b, k, :],
                            vi[:, b // 2, l0 : l0 + SUB // NPC, k, :],
                            start=(k == 0),
                            stop=(k == 1),
                        )
                b = (t * PT) // cpb
                l0 = ((t * PT) % cpb) // NPC
                epi(
                    vo[:, b, l0 : l0 + PT // NPC, :],
                    pt[:, :],
                    b_sb[:, b : b + 1],
                    cols=PT,
                )

        v1 = act_pool.tile([C, NCOL], BF16, tag="act", name="v1")
        s0_tiles((0, 1))
        for g in range(4):
            if g < 3:
                s0_tiles((2 * g + 2, 2 * g + 3))
            down_level(v0, v1, w1, b1, 2, 32, tiles=(g, g + 4))
        v2 = act_pool.tile([C, NCOL], BF16, tag="act", name="v2")
        down_level(v1, v2, w2, b2, 4, 16)
        v3 = act_pool.tile([C, NCOL], BF16, tag="act", name="v3")
        down_level(v2, v3, w3, b3, 8, 8)

        # ---------------- middle switch ----------------
        # v3: (c, [itk=8, itx=8, n]); vm: (c, [itx=8, itk=8, n])
        # Per-(itx,itk) bias seeded into PSUM by a K=4 indicator matmul,
        # then the per-block c x c matmuls accumulate on top.
        vm = act_pool.tile([C, NCOL], BF16, tag="act", name="vm")
        v3v = v3.rearrange("p (k x n) -> p k x n", k=8, x=8, n=NPC)
        wmv = wm.rearrange("p (k x d) -> p k x d", k=8, x=8, d=C)
        for t in range(NCOL // PT):  # tile t covers itx = t
            pt = psum_pool.tile([C, PT], F32, tag="pt", name="pm")
            if t % 2 == 0:
                # seed per-block biases into PSUM with K=4 indicator matmuls,
                # then accumulate the per-(itk,itx) c x c matmuls on top; the
                # epilogue is then a plain relu (greedy engine choice).
                for sgrp in range(2):
                    nc.tensor.matmul(
                        pt[:, sgrp * SUB : (sgrp + 1) * SUB],
                        mb2[:, (2 * t + sgrp) * C : (2 * t + sgrp + 1) * C],
                        ind[:, :],
                        start=True,
                        stop=False,
                        skip_group_check=True,
                    )
                    for bi in range(4):
                        blk = 4 * sgrp + bi
                        nc.tensor.matmul(
                            pt[:, blk * NPC : (blk + 1) * NPC],
                            wmv[:, blk, t, :],
                            v3v[:, blk, t, :],
                            start=False,
                            stop=(bi == 3),
                            skip_group_check=True,
                        )
                epi(vm[:, t * PT : (t + 1) * PT], pt[:, :], None, cols=PT)
            else:
                for blk in range(8):  # block within tile (= itk); global = 8t + blk
                    nc.tensor.matmul(
                        pt[:, blk * NPC : (blk + 1) * NPC],
                        wmv[:, blk, t, :],
                        v3v[:, blk, t, :],
                        start=True,
                        stop=True,
                    )
                # TT-add (V) with a broadcast bias view + in-place Scalar relu
                ptv = pt.rearrange("p (b n) -> p b n", b=8, n=NPC)
                bias_v = mb[:, 8 * t : 8 * (t + 1)].unsqueeze(2).broadcast_to((C, 8, NPC))
                dst = vm[:, t * PT : (t + 1) * PT]
                dstv = dst.rearrange("p (b n) -> p b n", b=8, n=NPC)
                nc.vector.tensor_tensor(dstv, ptv, bias_v, ALU.add)
                load_ns["v"] += (120 + PT) / 0.96
                nc.scalar.activation(dst, dst, AF.Relu)
                load_ns["s"] += (352 + PT) / 1.2

        # ---------------- up levels 4..6 ----------------
        def up_level(vin, vout, w_sb, b_sb, nb_in, l_in, tiles=None):
            """vin: (c, [x=nb_in, l_in, n]); vout: (c, [xo=nb_in/2, 2*l_in, n]);
            vout[:, xo, 2*l+j, :] = relu(sum_k vin[:, 2xo+k, l, :] @ W[xo,j,k] + B[xo,j])."""
            nbo = nb_in // 2
            wv = w_sb.rearrange("p (x j k d) -> p x j k d", x=nbo, j=2, k=2, d=C)
            vi = vin.rearrange("p (x l n) -> p x l n", x=nb_in, l=l_in, n=NPC)
            vo = vout.rearrange("p (x l j n) -> p x l j n", x=nbo, l=l_in, j=2, n=NPC)
            cpb = l_in * NPC  # columns per (xo, j) output block
            for t in tiles if tiles is not None else range(NCOL // PT):
                pt = psum_pool.tile([C, PT], F32, tag="pt", name="pu")
                for k in range(2):
                    for s in range(2):
                        col = t * PT + s * SUB
                        g = col // cpb  # global (xo, j) block index, j-minor
                        xo, j = g // 2, g % 2
                        lt0 = (col % cpb) // NPC
                        nc.tensor.matmul(
                            pt[:, s * SUB : (s + 1) * SUB],
                            wv[:, xo, j, k, :],
                            vi[:, 2 * xo + k, lt0 : lt0 + SUB // NPC, :],
                            start=(k == 0),
                            stop=(k == 1),
                        )
                g = (t * PT) // cpb
                xo, j = g // 2, g % 2
                lt0 = ((t * PT) % cpb) // NPC
                epi(
                    vo[:, xo, lt0 : lt0 + PT // NPC, j, :],
                    pt[:, :],
                    b_sb[:, 2 * xo + j : 2 * xo + j + 1],
                    cols=PT,
                )

        v4 = act_pool.tile([C, NCOL], BF16, tag="act", name="v4")
        up_level(vm, v4, w4, b4, 8, 8)
        v5 = act_pool.tile([C, NCOL], BF16, tag="act", name="v5")
        up_level(v4, v5, w5, b5, 4, 16)
        v6 = act_pool.tile([C, NCOL], BF16, tag="act", name="v6")

        # ---------------- output conv (no bias / relu), interleaved with L6 --
        done_out = [0]  # next OUT_CHUNKS index to store

        def out_tiles(ts):
            for t in ts:
                pt = psum_pool.tile([C, PT], F32, tag="pt", name="po")
                for s in range(2):
                    col = t * PT + s * SUB
                    nc.tensor.matmul(
                        pt[:, s * SUB : (s + 1) * SUB],
                        wkf[:, :],
                        v6[:, col : col + SUB],
                        start=True,
                        stop=True,
                    )
                epi(yo[:, t * PT : (t + 1) * PT], pt[:, :], None, relu=False, cols=PT)
                # issue chunked output stores as soon as their columns are done
                while done_out[0] < len(OUT_CHUNKS) and OUT_CHUNKS[done_out[0]][1] <= (t + 1) * PT:
                    a, b = OUT_CHUNKS[done_out[0]]
                    nc.sync.dma_start(out=out_d[:, a:b], in_=yo[:, a:b])
                    done_out[0] += 1

        # L6 j=0 tiles are 0..3, j=1 tiles are 4..7 (cpb=4096); out tile pair
        # (2q, 2q+1) needs quarter q of both j streams.  Run one quarter ahead
        # so out-tile matmuls never wait on a just-finished L6 epilogue.
        up_level(v5, v6, w6, b6, 2, 32, tiles=(0, 4))
        for q in range(3):
            up_level(v5, v6, w6, b6, 2, 32, tiles=(q + 1, 5 + q))
            out_tiles((2 * q, 2 * q + 1))
        out_tiles((6, 7))

    nc.finalize()
    return nc


_NC_CACHE = {}


def _get_nc():
    if "nc" not in _NC_CACHE:
        _NC_CACHE["nc"] = build_nc()
    return _NC_CACHE["nc"]


def _prep_in_maps(inputs):
    x = np.asarray(inputs["x"], np.float32)
    bf = lambda a: np.ascontiguousarray(np.asarray(a, np.float32)).astype(ml_dtypes.bfloat16)
    f32 = lambda a: np.ascontiguousarray(np.asarray(a, np.float32))
    mbv = np.asarray(inputs["mb"], np.float32)  # (k=8, x=8, c)
    mbT = mbv.transpose(1, 0, 2).reshape(64, C).T  # (c, 64), col = x*8 + k
    wmat = lambda key, nb: np.asarray(inputs[key], np.float32).reshape(nb, C, C).transpose(1, 0, 2).reshape(C, nb * C)
    wb = np.concatenate(
        [
            np.asarray(inputs["xf"], np.float32),  # (f=128, c) as lhsT directly
            wmat("f1", 4),
            wmat("f2", 8),
            wmat("f3", 16),
            np.asarray(inputs["md"], np.float32).reshape(64, C, C).transpose(1, 0, 2).reshape(C, 64 * C),
            wmat("f4", 16),
            wmat("f5", 8),
            wmat("f6", 4),
            np.asarray(inputs["kf"], np.float32),
        ],
        axis=1,
    )
    bia = np.concatenate(
        [
            np.asarray(inputs["xb"], np.float32).reshape(C, 1),
            np.asarray(inputs["b1"], np.float32).T,
            np.asarray(inputs["b2"], np.float32).T,
            np.asarray(inputs["b3"], np.float32).T,
            np.asarray(inputs["b4"], np.float32).T,
            np.asarray(inputs["b5"], np.float32).T,
            np.asarray(inputs["b6"], np.float32).T,
            mbT,
        ],
        axis=1,
    )
    # mid-bias lhsT slices: u = 2*t + sgrp (t = itx tile, sgrp = 512-col half);
    # row ki covers block k = 4*sgrp + ki at x = t: mb2[ki, u*C+d] = mb[4*(u%2)+ki, u//2, d]
    mbind = np.zeros((4, 20 * C), np.float32)
    for u in range(16):
        t_, sgrp = u // 2, u % 2
        for ki in range(4):
            mbind[ki, u * C : (u + 1) * C] = mbv[4 * sgrp + ki, t_, :]
    for ki in range(4):
        mbind[ki, 16 * C + ki * NPC : 16 * C + (ki + 1) * NPC] = 1.0
    shared = {
        "wb": bf(wb),
        "bia": f32(bia),
        "mbind": bf(mbind),
    }
    in_maps = []
    for i in range(N_CORES):
        xs = x[i * NPC : (i + 1) * NPC]  # (128, 8192)
        xt = (
            np.ascontiguousarray(xs.reshape(NPC, 64, C).transpose(2, 1, 0))
            .reshape(C, NCOL)
            .astype(ml_dtypes.bfloat16)
        )
        in_maps.append({"xt": xt, **shared})
    return in_maps


def _gather(results):
    outs = []
    for i in range(N_CORES):
        r = np.asarray(results[i]["out"]).astype(np.float32)  # (C=k_out, [l=64, n=128])
        outs.append(r.reshape(C, 64, NPC).transpose(2, 1, 0).reshape(NPC, 64 * C))
    return np.concatenate(outs, axis=0).astype(np.float32)


def _enable_ntff_hook():
    """Register the axon NTFF profiling hook (missing from this image's
    antenv) so run_bass_kernel_spmd(trace=True) can measure HW exec time."""
    import types

    if "antenv.axon_hooks" in sys.modules:
        return
    import antenv
    from trn_agent_boot.trn_boot import _ntff_profile_via_ctypes

    hook = _ntff_profile_via_ctypes("/opt/axon/libaxon_pjrt.so")
    mod = types.ModuleType("antenv.axon_hooks")
    mod.get_axon_ntff_profile_hook = lambda: hook
    mod.set_axon_ntff_profile_hook = lambda h: None
    sys.modules["antenv.axon_hooks"] = mod
    antenv.axon_hooks = mod
    import concourse.bass_utils as bu

    bu.upload_artifacts = lambda tmpdir: tmpdir  # keep artifacts local


def run(inputs, trace=False, **kw):
    nc = _get_nc()
    in_maps = _prep_in_maps(inputs)
    if trace:
        _enable_ntff_hook()
    res = run_bass_kernel_spmd(nc, in_maps, core_ids=list(range(N_CORES)), trace=trace, **kw)
    return _gather(res.results), res


def kernel(**inputs) -> np.ndarray:
    out, _ = run(inputs, trace=False)
    return out
